# revision 70
# baseline (speedup 1.0000x reference)
"""Trainium2 Bass kernel for a 3-layer GraphSAGE GNN (CellTypeGNN).

Strategy (graph/data parallel over 8 NeuronCores):
- Nodes are sharded by range: core c owns nodes [c*6250, (c+1)*6250).
- Edges are assigned to the core owning their destination node, grouped into
  128-node destination windows, and packed into 128-edge subtiles.
- Messages x[src] are fetched with batched SWDGE dma_gather (fp16, 256B
  rows) from a full replica of x in DRAM in 16-subtile (2048-descriptor)
  chunks, round-robined over 4 SWDGE queues. The Q7 is held for roughly a
  chunk's SDMA drain, so chunk size trades per-chunk fixed cost against
  hold granularity; G=16 measured best. int16 gather indices only reach
  32767, so x is split into lo (nodes < 25000) and hi tables; edges are
  segregated by source half within each window. Gather indices are
  layer-invariant and preloaded into SBUF once.
- Segment-mean aggregation: fp16 one-hot scatter matrices (scaled by
  1/deg(dst)), precomputed on the host, streamed from DRAM; PE matmuls
  msg^T @ onehot accumulate the mean into PSUM per destination window,
  feature-major.
- The whole post-aggregation pipeline stays FEATURE-major (features on
  partitions, nodes on the free dim) -- no per-window transposes:
  * LN stats per node via PE: ones^T @ [h | h^2] -> [1, 2*128] PSUM strip.
  * rstd via Newton-rsqrt on DVE (bit-hack seed + 2 iterations) so the
    Scalar engine only ever uses the gelu_and_others ACT table set
    (Identity/Square/Gelu) -- zero ACT table reloads in steady state.
  * per-node affine (a=rstd, b2=-mu*rstd) broadcast across partitions with
    a K=1 PE matmul; normalize with two DVE tensor_tensor ops.
  * LN affine + GELU fused in one scalar-engine activation (per-partition
    scale/bias = per-feature g/b).
- Node features kept in fp16 (residuals in fp16); per-window PE transpose +
  DMA writes the node-major fp16 rows for the next layer's gather.
- After layers 1 and 2 the updated fp16 node features are AllGathered.
- Classifier (LN + linear, LN affine folded into the weights on the host)
  is fused into layer 3's window loop.
"""

import numpy as np
from contextlib import ExitStack

import concourse.bass as bass
import concourse.tile as tile
from concourse import bacc, mybir
from concourse.bass_utils import run_bass_kernel_spmd

P = 128
N, E, D = 50000, 800000, 128
DOUT, NCLS = 64, 40
NCORES = 8
NPC = N // NCORES            # 6250 nodes per core
W = (NPC + P - 1) // P       # 49 windows per core
NPAD = W * P                 # 6272 padded per-core node count
HALF = 25000                 # lo/hi table split (int16 gather indices)
G = 20                       # gather chunk size in 128-edge subtiles
LN_EPS = 1e-5
RSQRT_MAGIC = 0x5F3759DF
PREP = False                 # prepare_only + trigger_dma gather mode

f32 = mybir.dt.float32
f16 = mybir.dt.float16
i16 = mybir.dt.int16
i32 = mybir.dt.int32

_cache = {}


def _schedule(edge_index):
    """Host-side edge preprocessing. Returns per-core gather/one-hot arrays
    plus the SPMD-uniform window subtile schedule."""
    src = edge_index[0].astype(np.int64)
    dst = edge_index[1].astype(np.int64)
    deg = np.bincount(dst, minlength=N)
    invdeg_all = (1.0 / np.maximum(deg, 1)).astype(np.float32)

    core = dst // NPC
    loc = dst - core * NPC
    win = loc >> 7
    rel = (loc & 127).astype(np.float32)
    half = (src >= HALF).astype(np.int64)

    # group id: (core, window, half); edges sorted by group
    gid = (core * W + win) * 2 + half
    order = np.argsort(gid, kind="stable")
    gid_s = gid[order]
    counts = np.bincount(gid_s, minlength=NCORES * W * 2).reshape(NCORES, W, 2)

    # SPMD-uniform subtile counts per (window, half)
    ntiles = (counts.max(axis=0) + P - 1) // P  # [W, 2]
    empty = ntiles.sum(axis=1) == 0
    ntiles[empty, 0] = 1
    TL = int(ntiles[:, 0].sum())
    TH = int(ntiles[:, 1].sum())
    startA = np.concatenate([[0], np.cumsum(ntiles[:, 0])[:-1]]).astype(np.int64)
    startB = np.concatenate([[0], np.cumsum(ntiles[:, 1])[:-1]]).astype(np.int64)

    # per-core stream arrays
    idxA = np.zeros((NCORES, P, TL), np.int16)
    idxB = np.zeros((NCORES, P, TH), np.int16)
    relA = np.full((NCORES, P, TL), -1.0, np.float32)
    relB = np.full((NCORES, P, TH), -1.0, np.float32)
    invA = np.zeros((NCORES, P, TL), np.float32)
    invB = np.zeros((NCORES, P, TH), np.float32)

    # vectorized placement: rank of each edge within its (core, win, half) group
    grp_start_per_edge = np.concatenate([[0], np.cumsum(np.bincount(
        gid_s, minlength=NCORES * W * 2))])[gid_s]
    rank = np.arange(len(gid_s)) - grp_start_per_edge
    e_core = core[order]
    e_win = win[order]
    e_half = half[order]
    e_src = src[order]
    e_rel = rel[order]
    e_inv = invdeg_all[dst[order]]
    pos = np.where(e_half == 0, startA[e_win], startB[e_win]) + (rank >> 7)
    prt = rank & 127

    mA = e_half == 0
    idxA[e_core[mA], prt[mA], pos[mA]] = e_src[mA].astype(np.int16)
    relA[e_core[mA], prt[mA], pos[mA]] = e_rel[mA]
    invA[e_core[mA], prt[mA], pos[mA]] = e_inv[mA]
    mB = ~mA
    idxB[e_core[mB], prt[mB], pos[mB]] = (e_src[mB] - HALF).astype(np.int16)
    relB[e_core[mB], prt[mB], pos[mB]] = e_rel[mB]
    invB[e_core[mB], prt[mB], pos[mB]] = e_inv[mB]

    def wrap(idx_pt):  # [P, T] slot-major -> wrapped [128, T*8] per chunk
        Tn = idx_pt.shape[1]
        out = np.zeros((128, Tn * 8), np.int16)
        for c0 in range(0, Tn, G):
            c1 = min(c0 + G, Tn)
            flat = idx_pt[:, c0:c1].T.reshape(-1)  # i = t_local*128 + p
            w16 = flat.reshape(-1, 16).T  # [16, n/16]: i -> [i%16, i//16]
            out[:, c0 * 8 : c0 * 8 + w16.shape[1]] = np.tile(w16, (8, 1))
        return out

    idxAw = np.stack([wrap(idxA[c]) for c in range(NCORES)])
    idxBw = np.stack([wrap(idxB[c]) for c in range(NCORES)])

    def onehot(rel, inv):
        # [NCORES, P, T] -> [NCORES, P, T*P] fp16: oh[c,p,t*P+j] =
        # inv[c,p,t] * (j == rel[c,p,t]); rel<0 slots stay zero.
        nc_, pp, tt = rel.shape
        oh = np.zeros((nc_, pp, tt, P), np.float16)
        ci, pi, ti = np.nonzero(rel >= 0)
        oh[ci, pi, ti, rel[ci, pi, ti].astype(np.int64)] = inv[ci, pi, ti]
        return oh.reshape(nc_, pp, tt * P)

    return dict(
        ntiles=ntiles, TL=TL, TH=TH, startA=startA, startB=startB,
        idxA=idxAw, idxB=idxBw,
        ohA=onehot(relA, invA), ohB=onehot(relB, invB),
    )


def _build(sched):
    """Build and compile the SPMD Bass program."""
    ntiles, TL, TH = sched["ntiles"], sched["TL"], sched["TH"]
    startA, startB = sched["startA"], sched["startB"]

    nc = bacc.Bacc("TRN2", target_bir_lowering=False, debug=False,
                   num_devices=NCORES, num_swdge_queues=4,
                   dynamic_dma_scratch_size=65536)

    def din(name, shape, dt):
        return nc.dram_tensor(name, shape, dt, kind="ExternalInput")

    xf16_d = din("xf16", [N, D], f16)
    xfm16_d = din("xfm16", [P, NPAD], f16)
    idxA_d = din("idxA", [P, TL * 8], i16)
    idxB_d = din("idxB", [P, TH * 8], i16)
    ohA_d = din("ohA", [P, TL * P], f16)
    ohB_d = din("ohB", [P, TH * P], f16)
    ident_d = din("ident", [P, P], f16)
    statsel_d = din("statsel", [P, 64], f16)   # stats row-routing lhsT
    selbank_d = din("selbank", [P, 8 * P], f16)  # broadcast row-select lhsT
    magic_d = din("magic", [P, P], i32)     # rsqrt bit-hack constant
    c15_d = din("c15", [P, P], f32)         # Newton constant 1.5
    wl_d = [din(f"wl{l}", [D, D if l < 2 else DOUT], f16) for l in range(3)]
    wr_d = [din(f"wr{l}", [D, D if l < 2 else DOUT], f16) for l in range(3)]
    bl_d = [din(f"bl{l}", [D if l < 2 else DOUT, 1], f32) for l in range(3)]
    g_d = [din(f"g{l}", [D, 1], f32) for l in range(2)]
    b_d = [din(f"b{l}", [D, 1], f32) for l in range(2)]
    wc_d = din("wc", [DOUT, NCLS], f16)
    bc_d = din("bc", [NCLS, 1], f32)
    out_d = nc.dram_tensor("out", [NCLS, NPAD], f32, kind="ExternalOutput")

    xg_own = [nc.dram_tensor(f"xg{l}_own", [NPC, D], f16) for l in range(2)]
    xg_full = [
        nc.dram_tensor(f"xg{l}_full", [N, D], f16, addr_space="Shared")
        for l in range(2)
    ]

    with tile.TileContext(nc) as tc, ExitStack() as ctx:
        cpool = ctx.enter_context(tc.tile_pool(name="const", bufs=1))
        xpool = ctx.enter_context(tc.tile_pool(name="x", bufs=1))
        msgApool = ctx.enter_context(tc.tile_pool(name="msgA", bufs=3))
        msgBpool = ctx.enter_context(tc.tile_pool(name="msgB", bufs=3))
        ohApool = ctx.enter_context(tc.tile_pool(name="ohA", bufs=3))
        ohBpool = ctx.enter_context(tc.tile_pool(name="ohB", bufs=3))
        wkpool = ctx.enter_context(tc.tile_pool(name="wk", bufs=4))
        hsqpool = ctx.enter_context(tc.tile_pool(name="hsq", bufs=17))
        stpool = ctx.enter_context(tc.tile_pool(name="st", bufs=4))
        psA = ctx.enter_context(tc.tile_pool(name="psA", bufs=2, space="PSUM"))
        psH = ctx.enter_context(tc.tile_pool(name="psH", bufs=2, space="PSUM"))
        psS = ctx.enter_context(tc.tile_pool(name="psS", bufs=1, space="PSUM"))
        psB = ctx.enter_context(tc.tile_pool(name="psB", bufs=1, space="PSUM"))
        psT = ctx.enter_context(tc.tile_pool(name="psT", bufs=1, space="PSUM"))

        def load(dram, shape, dt):
            t = cpool.tile(shape, dt, name=f"c_{dram.name}")
            nc.sync.dma_start(out=t[:], in_=dram.ap())
            return t

        ident_t = load(ident_d, [P, P], f16)
        statsel_t = load(statsel_d, [P, 64], f16)
        selbank_t = load(selbank_d, [P, 8 * P], f16)
        magic_t = load(magic_d, [P, P], i32)
        c15_t = load(c15_d, [P, P], f32)
        wl_t = [load(wl_d[l], [D, D if l < 2 else DOUT], f16) for l in range(3)]
        wr_t = [load(wr_d[l], [D, D if l < 2 else DOUT], f16) for l in range(3)]
        bl_t = [load(bl_d[l], [D if l < 2 else DOUT, 1], f32) for l in range(3)]
        g_t = [load(g_d[l], [D, 1], f32) for l in range(2)]
        b_t = [load(b_d[l], [D, 1], f32) for l in range(2)]
        wc_t = load(wc_d, [DOUT, NCLS], f16)
        bc_t = load(bc_d, [NCLS, 1], f32)
        # layer-invariant gather indices, preloaded whole
        idxA_t = load(idxA_d, [P, TL * 8], i16)
        idxB_t = load(idxB_d, [P, TH * 8], i16)

        xfm16 = [xpool.tile([P, NPAD], f16, tag=f"xfm16_{i}", name=f"xfm16_{i}")
                 for i in range(2)]
        nc.sync.dma_start(out=xfm16[0][:], in_=xfm16_d.ap())

        qctr = [0]
        dma_sems = [nc.alloc_semaphore(f"swdge_dma{q}") for q in range(4)]

        for l in range(3):
            dout = D if l < 2 else DOUT
            cur16 = xfm16[l % 2]
            nxt16 = xfm16[(l + 1) % 2]
            if l == 0:
                src_lo = xf16_d.ap()[:HALF, :]
                src_hi = xf16_d.ap()[HALF:, :]
            else:
                src_lo = xg_full[l - 1].ap()[:HALF, :]
                src_hi = xg_full[l - 1].ap()[HALF:, :]

            # emit gather chunks lazily; Tile pool backpressure pipelines them
            msgs = {"A": {}, "B": {}}
            ohs = {"A": {}, "B": {}}
            issued = {"A": -1, "B": -1}

            def emit_chunk(stream, ci, src_lo=src_lo, src_hi=src_hi,
                           msgs=msgs, ohs=ohs):
                Tn = TL if stream == "A" else TH
                idxt = idxA_t if stream == "A" else idxB_t
                ohd = ohA_d if stream == "A" else ohB_d
                mpool = msgApool if stream == "A" else msgBpool
                opool = ohApool if stream == "A" else ohBpool
                src = src_lo if stream == "A" else src_hi
                c0 = ci * G
                cn = min(G, Tn - c0)
                nidx = cn * P
                q = qctr[0] % 4
                mt = mpool.tile([P, G * P], f16, tag=f"msg{stream}")
                nc.gpsimd.dma_gather(
                    mt[:, : cn * P].rearrange("p (t d) -> p t d", d=P),
                    src,
                    idxt[:, c0 * 8 : c0 * 8 + cn * 8],
                    nidx,
                    nidx,
                    P,
                    single_packet=False,
                    queue_num=q,
                )
                qctr[0] += 1
                ot = opool.tile([P, G * P], f16, tag=f"oh{stream}")
                nc.sync.dma_start(
                    out=ot[:, : cn * P],
                    in_=ohd.ap()[:, c0 * P : (c0 + cn) * P],
                )
                msgs[stream][ci] = mt
                ohs[stream][ci] = ot

            def phase1(w, blk_i, kb, sps_blk, hsqs):
                """agg + SAGE linear + [h | h^2] + stats matmul for window w."""
                nA, nB = int(ntiles[w, 0]), int(ntiles[w, 1])
                subs = [("A", int(startA[w]) + i) for i in range(nA)] + [
                    ("B", int(startB[w]) + i) for i in range(nB)
                ]
                for stream, pos in subs:
                    while issued[stream] < pos // G:
                        issued[stream] += 1
                        emit_chunk(stream, issued[stream])

                cols = slice(w * P, (w + 1) * P)
                ps = psA.tile([P, P], f32, space="PSUM", tag="agg")
                for si, (stream, pos) in enumerate(subs):
                    mt = msgs[stream][pos // G]
                    ot = ohs[stream][pos // G]
                    t = pos % G
                    nc.tensor.matmul(
                        out=ps[:],
                        lhsT=mt[:, t * P : (t + 1) * P],
                        rhs=ot[:, t * P : (t + 1) * P],
                        start=(si == 0),
                        stop=(si == len(subs) - 1),
                    )
                agg16 = wkpool.tile([P, P], f16, tag="agg16")
                nc.any.tensor_copy(agg16[:], ps[:])

                hps = psH.tile([dout, P], f32, space="PSUM", tag="h")
                nc.tensor.matmul(out=hps[:], lhsT=wl_t[l][:], rhs=agg16[:],
                                 start=True, stop=False)
                nc.tensor.matmul(out=hps[:], lhsT=wr_t[l][:],
                                 rhs=cur16[:, cols],
                                 start=False, stop=True)

                # hsq = [h | h^2] fp16; ACT stays in the gelu_and_others
                # table set (Identity/Square/Gelu) -- no table reloads.
                hsq = hsqpool.tile([dout, 2 * P], f16, tag="hsq")
                fn1 = (mybir.ActivationFunctionType.Identity if l < 2
                       else mybir.ActivationFunctionType.Gelu)
                nc.scalar.activation(hsq[:, :P], hps[:], fn1,
                                     bias=bl_t[l][:, :1])
                nc.scalar.activation(hsq[:, P:], hsq[:, :P],
                                     mybir.ActivationFunctionType.Square)
                # route this window's [sum | sumsq] into block row blk_i:
                # lhsT column j is all-ones iff j == blk_i
                nc.tensor.matmul(out=sps_blk[:],
                                 lhsT=statsel_t[:dout, blk_i * 8 : blk_i * 8 + 8],
                                 rhs=hsq[:], start=(blk_i == 0),
                                 stop=(blk_i == kb - 1))
                hsqs[w] = hsq

            def strip_math(kb, sps_blk):
                """per-node LN stats -> [a | b2] for a block of kb windows.
                All fast-path DVE ops (single float scalars / tensor-tensor);
                rstd via bit-hack seed + 2 Newton iterations."""
                invd = 1.0 / dout
                sl = slice(0, kb)
                ex2 = stpool.tile([kb, P], f32, tag="ex2")
                nc.vector.tensor_scalar_mul(ex2[:], sps_blk[sl, P:], invd)
                negmu = stpool.tile([kb, P], f32, tag="negmu")
                nc.vector.tensor_scalar_mul(negmu[:], sps_blk[sl, :P], -invd)
                musq = stpool.tile([kb, P], f32, tag="musq")
                nc.vector.tensor_mul(musq[:], negmu[:], negmu[:])
                musqe = stpool.tile([kb, P], f32, tag="musqe")
                nc.vector.tensor_scalar_sub(musqe[:], musq[:], LN_EPS)
                vare = stpool.tile([kb, P], f32, tag="vare")
                nc.vector.tensor_sub(vare[:], ex2[:], musqe[:])
                ishr = stpool.tile([kb, P], i32, tag="ishr")
                nc.vector.tensor_scalar(
                    ishr[:], vare[:].bitcast(i32), 1, None,
                    op0=mybir.AluOpType.logical_shift_right)
                y0 = stpool.tile([kb, P], i32, tag="y0")
                nc.vector.tensor_sub(y0[:], magic_t[:kb, :P], ishr[:])
                halfv = stpool.tile([kb, P], f32, tag="halfv")
                nc.vector.tensor_scalar_mul(halfv[:], vare[:], 0.5)
                ab = stpool.tile([kb, 2 * P], f16, tag="ab")
                ycur = y0[:].bitcast(f32)
                for it in range(2):
                    yy = stpool.tile([kb, P], f32, tag=f"yy{it}")
                    nc.vector.tensor_mul(yy[:], ycur, ycur)
                    t_ = stpool.tile([kb, P], f32, tag=f"t{it}")
                    nc.vector.tensor_mul(t_[:], halfv[:], yy[:])
                    s_ = stpool.tile([kb, P], f32, tag=f"s{it}")
                    nc.vector.tensor_sub(s_[:], c15_t[:kb, :P], t_[:])
                    if it == 0:
                        yn = stpool.tile([kb, P], f32, tag="y1")
                        nc.vector.tensor_mul(yn[:], ycur, s_[:])
                        ycur = yn[:]
                    else:
                        nc.vector.tensor_mul(ab[:, :P], ycur, s_[:])
                nc.vector.tensor_mul(ab[:, P:], negmu[:], ab[:, :P])
                return ab

            def phase3(w, blk_i, kb, ab, hsqs):
                """broadcast + normalize + gelu (+ residual / classifier)."""
                cols = slice(w * P, (w + 1) * P)
                hsq = hsqs.pop(w)
                bps = psB.tile([dout, 2 * P], f32, space="PSUM", tag="bc")
                nc.tensor.matmul(out=bps[:],
                                 lhsT=selbank_t[:kb, blk_i * P : blk_i * P + dout],
                                 rhs=ab[:kb, :],
                                 start=True, stop=True)
                t1 = wkpool.tile([dout, P], f32, tag="t1")
                nc.vector.tensor_mul(t1[:], hsq[:, :P], bps[:, :P])
                if l < 2:
                    t2 = wkpool.tile([dout, P], f32, tag="t2")
                    nc.vector.tensor_add(t2[:], t1[:], bps[:, P:])
                    gel16 = wkpool.tile([dout, P], f16, tag="gel16")
                    nc.scalar.activation(gel16[:], t2[:],
                                         mybir.ActivationFunctionType.Gelu,
                                         bias=b_t[l][:, :1],
                                         scale=g_t[l][:, :1])
                    nc.vector.tensor_add(nxt16[:, cols], gel16[:],
                                         cur16[:, cols])
                    tp = psT.tile([P, P], f16, space="PSUM", tag="tp")
                    nc.tensor.transpose(tp[:], nxt16[:, cols], ident_t[:])
                    xnm = wkpool.tile([P, P], f16, tag="xnm")
                    nc.any.tensor_copy(xnm[:], tp[:])
                    rows = min(P, NPC - w * P)
                    nc.sync.dma_start(
                        out=xg_own[l].ap()[w * P : w * P + rows, :],
                        in_=xnm[:rows, :],
                    )
                else:
                    norm16 = wkpool.tile([dout, P], f16, tag="norm16")
                    nc.vector.tensor_add(norm16[:], t1[:], bps[:, P:])
                    ops_ = psB.tile([NCLS, P], f32, space="PSUM", tag="bc")
                    nc.tensor.matmul(out=ops_[:], lhsT=wc_t[:],
                                     rhs=norm16[:], start=True, stop=True)
                    osb = wkpool.tile([NCLS, P], f32, tag="osb")
                    nc.scalar.activation(osb[:], ops_[:],
                                         mybir.ActivationFunctionType.Identity,
                                         bias=bc_t[:, :1])
                    nc.sync.dma_start(out=out_d.ap()[:, cols], in_=osb[:])

            # software-pipelined blocks: phase1(b), phase3(b-1), strip(b)
            BK = 8
            blocks = [list(range(b, min(b + BK, W))) for b in range(0, W, BK)]
            hsqs = {}
            prev = None  # (wins, ab)
            for wins in blocks:
                kb = len(wins)
                sps_blk = psS.tile([BK, 2 * P], f32, space="PSUM", tag="st")
                for i, w in enumerate(wins):
                    phase1(w, i, kb, sps_blk, hsqs)
                if prev is not None:
                    pwins, pab = prev
                    for i, w in enumerate(pwins):
                        phase3(w, i, len(pwins), pab, hsqs)
                prev = (wins, strip_math(kb, sps_blk))
            pwins, pab = prev
            for i, w in enumerate(pwins):
                phase3(w, i, len(pwins), pab, hsqs)

            if l < 2:
                nc.gpsimd.collective_compute(
                    "AllGather",
                    mybir.AluOpType.bypass,
                    replica_groups=[list(range(NCORES))],
                    ins=[xg_own[l].ap()],
                    outs=[xg_full[l].ap()],
                )

    nc.compile()
    return nc


def _statsel():
    # statsel[p, i*8 + j] = 1 iff j == i: lhsT slice [dout, 8] for window i
    # has column i all-ones -> stats land in block row i.
    s = np.zeros((P, 64), np.float16)
    for i in range(8):
        s[:, i * 8 + i] = 1.0
    return s


def _selbank():
    # selbank[j, i*P + f] = 1 iff j == i: lhsT slice [kb, dout] for window i
    # selects block-strip row i and broadcasts it across all dout partitions.
    s = np.zeros((P, 8 * P), np.float16)
    for i in range(8):
        s[i, i * P : (i + 1) * P] = 1.0
    return s


def _prep_inputs(x, sched, weights):
    """Build per-core input maps."""
    xf16 = x.astype(np.float16)
    (Wl1, bl1, Wr1, g1, b1, Wl2, bl2, Wr2, g2, b2,
     Wl3, bl3, Wr3, gc, bc, Wc, bcls) = weights
    wcp = (gc[:, None].astype(np.float32) * Wc.astype(np.float32))
    bcp = bc.astype(np.float32) @ Wc.astype(np.float32) + bcls.astype(np.float32)
    common = {
        "xf16": xf16,
        "ident": np.eye(P, dtype=np.float16),
        "statsel": _statsel(),
        "selbank": _selbank(),
        "magic": np.full((P, P), RSQRT_MAGIC, np.int32),
        "c15": np.full((P, P), 1.5, np.float32),
        "wl0": Wl1.astype(np.float16), "wr0": Wr1.astype(np.float16),
        "wl1": Wl2.astype(np.float16), "wr1": Wr2.astype(np.float16),
        "wl2": Wl3.astype(np.float16), "wr2": Wr3.astype(np.float16),
        "bl0": bl1.reshape(-1, 1).astype(np.float32),
        "bl1": bl2.reshape(-1, 1).astype(np.float32),
        "bl2": bl3.reshape(-1, 1).astype(np.float32),
        "g0": g1.reshape(-1, 1).astype(np.float32),
        "b0": b1.reshape(-1, 1).astype(np.float32),
        "g1": g2.reshape(-1, 1).astype(np.float32),
        "b1": b2.reshape(-1, 1).astype(np.float32),
        "wc": wcp.astype(np.float16),
        "bc": bcp.reshape(-1, 1).astype(np.float32),
    }
    in_maps = []
    for c in range(NCORES):
        xc_ = x[c * NPC : (c + 1) * NPC].astype(np.float16)
        xfm = np.zeros((P, NPAD), np.float16)
        xfm[:, :NPC] = xc_.T
        m = dict(common)
        m.update(
            xfm16=xfm,
            idxA=sched["idxA"][c],
            idxB=sched["idxB"][c],
            ohA=sched["ohA"][c],
            ohB=sched["ohB"][c],
        )
        in_maps.append(m)
    return in_maps




class _Runner:
    """Persistent PJRT runner: traces/compiles once, keeps inputs on device,
    supports steady-state timing of repeated executions."""

    def __init__(self, nc, in_maps):
        import jax
        from jax.sharding import Mesh, PartitionSpec
        try:
            from jax.experimental.shard_map import shard_map
        except ImportError:
            from jax.shard_map import shard_map
        from concourse import bass2jax, mybir as mb

        bass2jax.install_neuronx_cc_hook()
        self.jax = jax
        partition_name = (
            nc.partition_id_tensor.name if nc.partition_id_tensor else None
        )
        in_names, out_names, out_avals, zero_outs = [], [], [], []
        for alloc in nc.m.functions[0].allocations:
            if not isinstance(alloc, mb.MemoryLocationSet):
                continue
            name = alloc.memorylocations[0].name
            if alloc.kind == "ExternalInput":
                if name != partition_name:
                    in_names.append(name)
            elif alloc.kind == "ExternalOutput":
                out_names.append(name)
                shape = tuple(alloc.tensor_shape)
                dtype = mb.dt.np(alloc.dtype)
                out_avals.append(jax.core.ShapedArray(shape, dtype))
                zero_outs.append(np.zeros(shape, dtype))
        n_params = len(in_names)
        all_names = in_names + out_names
        if partition_name is not None:
            all_names.append(partition_name)

        def _body(*args):
            operands = list(args)
            if partition_name is not None:
                operands.append(bass2jax.partition_id_tensor())
            outs = bass2jax._bass_exec_p.bind(
                *operands,
                out_avals=tuple(out_avals),
                in_names=tuple(all_names),
                out_names=tuple(out_names),
                lowering_input_output_aliases=(),
                sim_require_finite=True,
                sim_require_nnan=True,
                nc=nc,
            )
            return tuple(outs)

        devices = jax.devices()[:NCORES]
        mesh = Mesh(np.asarray(devices), ("core",))
        n_outs = len(out_avals)
        self.fn = jax.jit(
            shard_map(
                _body,
                mesh=mesh,
                in_specs=(PartitionSpec("core"),) * (n_params + n_outs),
                out_specs=(PartitionSpec("core"),) * n_outs,
                check_rep=False,
            ),
            keep_unused=True,
        )
        self.out_names = out_names
        self.out_avals = out_avals
        concat_in = [
            np.concatenate([np.asarray(in_maps[c][nm]) for c in range(NCORES)])
            for nm in in_names
        ]
        concat_zeros = [
            np.concatenate([z] * NCORES, axis=0) for z in zero_outs
        ]
        self.dev_args = [jax.device_put(a) for a in concat_in + concat_zeros]
        self.update_idx = {nm: i for i, nm in enumerate(in_names)}
        self.in_names = in_names

    def refresh(self, in_maps):
        for nm in self.in_names:
            arr = np.concatenate(
                [np.asarray(in_maps[c][nm]) for c in range(NCORES)]
            )
            self.dev_args[self.update_idx[nm]] = self.jax.device_put(arr)

    def update_input(self, name, per_core_arrays):
        arr = np.concatenate([np.asarray(a) for a in per_core_arrays])
        self.dev_args[self.update_idx[name]] = self.jax.device_put(arr)

    def run(self):
        outs = self.fn(*self.dev_args)
        self.jax.block_until_ready(outs)
        return [
            {
                nm: np.asarray(outs[i]).reshape(NCORES, *self.out_avals[i].shape)[c]
                for i, nm in enumerate(self.out_names)
            }
            for c in range(NCORES)
        ]

    def time(self, reps=20, warmup=2):
        import time as _time
        for _ in range(warmup):
            self.jax.block_until_ready(self.fn(*self.dev_args))
        t0 = _time.time()
        outs = None
        for _ in range(reps):
            outs = self.fn(*self.dev_args)
        self.jax.block_until_ready(outs)
        return (_time.time() - t0) / reps


def kernel(x, edge_index, Wl1, bl1, Wr1, g1, b1, Wl2, bl2, Wr2, g2, b2,
           Wl3, bl3, Wr3, gc, bc, Wc, bcls):
    x = np.asarray(x)
    edge_index = np.asarray(edge_index)
    runner = get_runner(x, edge_index, Wl1, bl1, Wr1, g1, b1, Wl2, bl2, Wr2,
                        g2, b2, Wl3, bl3, Wr3, gc, bc, Wc, bcls)
    results = runner.run()
    out = np.empty((N, NCLS), np.float32)
    for c in range(NCORES):
        out[c * NPC : (c + 1) * NPC] = results[c]["out"][:, :NPC].T
    return out


def get_runner(x, edge_index, Wl1, bl1, Wr1, g1, b1, Wl2, bl2, Wr2, g2, b2,
               Wl3, bl3, Wr3, gc, bc, Wc, bcls):
    x = np.asarray(x)
    edge_index = np.asarray(edge_index)
    sched = _schedule(edge_index)
    key = (sched["TL"], sched["TH"], tuple(sched["ntiles"].ravel().tolist()))
    if key not in _cache:
        _cache[key] = _build(sched)
    nc = _cache[key]
    weights = (Wl1, bl1, Wr1, g1, b1, Wl2, bl2, Wr2, g2, b2,
               Wl3, bl3, Wr3, gc, bc, Wc, bcls)
    in_maps = _prep_inputs(x, sched, [np.asarray(w) for w in weights])
    rkey = ("runner", key)
    if rkey not in _cache:
        _cache[rkey] = _Runner(nc, in_maps)
    else:
        _cache[rkey].refresh(in_maps)
    return _cache[rkey]


# revision 75
# speedup vs baseline: 1.1037x; 1.1037x over previous
"""Trainium2 Bass kernel for a 3-layer GraphSAGE GNN (CellTypeGNN).

Strategy (graph/data parallel over 8 NeuronCores):
- Nodes are sharded by range: core c owns nodes [c*6250, (c+1)*6250).
- Edges are assigned to the core owning their destination node, grouped into
  128-node destination windows, and packed into 128-edge subtiles.
- Messages x[src] are fetched with batched SWDGE dma_gather (fp16, 256B
  rows) from a full replica of x in DRAM in 16-subtile (2048-descriptor)
  chunks, round-robined over 4 SWDGE queues. The Q7 is held for roughly a
  chunk's SDMA drain, so chunk size trades per-chunk fixed cost against
  hold granularity; G=16 measured best. int16 gather indices only reach
  32767, so x is split into lo (nodes < 25000) and hi tables; edges are
  segregated by source half within each window. Gather indices are
  layer-invariant and preloaded into SBUF once.
- Segment-mean aggregation: fp16 one-hot scatter matrices (scaled by
  1/deg(dst)), precomputed on the host, streamed from DRAM; PE matmuls
  msg^T @ onehot accumulate the mean into PSUM per destination window,
  feature-major.
- The whole post-aggregation pipeline stays FEATURE-major (features on
  partitions, nodes on the free dim) -- no per-window transposes:
  * LN stats per node via PE: ones^T @ [h | h^2] -> [1, 2*128] PSUM strip.
  * rstd via Newton-rsqrt on DVE (bit-hack seed + 2 iterations) so the
    Scalar engine only ever uses the gelu_and_others ACT table set
    (Identity/Square/Gelu) -- zero ACT table reloads in steady state.
  * per-node affine (a=rstd, b2=-mu*rstd) broadcast across partitions with
    a K=1 PE matmul; normalize with two DVE tensor_tensor ops.
  * LN affine + GELU fused in one scalar-engine activation (per-partition
    scale/bias = per-feature g/b).
- Node features kept in fp16 (residuals in fp16); per-window PE transpose +
  DMA writes the node-major fp16 rows for the next layer's gather.
- After layers 1 and 2 the updated fp16 node features are AllGathered.
- Classifier (LN + linear, LN affine folded into the weights on the host)
  is fused into layer 3's window loop.
"""

import numpy as np
from contextlib import ExitStack

import concourse.bass as bass
import concourse.tile as tile
from concourse import bacc, mybir
from concourse.bass_utils import run_bass_kernel_spmd

P = 128
N, E, D = 50000, 800000, 128
DOUT, NCLS = 64, 40
NCORES = 8
NPC = N // NCORES            # 6250 nodes per core
W = (NPC + P - 1) // P       # 49 windows per core
NPAD = W * P                 # 6272 padded per-core node count
HALF = 25000                 # lo/hi table split (int16 gather indices)
G = 16                       # gather chunk size in 128-edge subtiles
LN_EPS = 1e-5
RSQRT_MAGIC = 0x5F3759DF
PREP = False                 # prepare_only + trigger_dma gather mode

f32 = mybir.dt.float32
f16 = mybir.dt.float16
i16 = mybir.dt.int16
i32 = mybir.dt.int32

_cache = {}


def _schedule(edge_index):
    """Host-side edge preprocessing. Returns per-core gather/one-hot arrays
    plus the SPMD-uniform window subtile schedule."""
    src = edge_index[0].astype(np.int64)
    dst = edge_index[1].astype(np.int64)
    deg = np.bincount(dst, minlength=N)
    invdeg_all = (1.0 / np.maximum(deg, 1)).astype(np.float32)

    core = dst // NPC
    loc = dst - core * NPC
    win = loc >> 7
    rel = (loc & 127).astype(np.float32)
    half = (src >= HALF).astype(np.int64)

    # group id: (core, window, half); edges sorted by group
    gid = (core * W + win) * 2 + half
    order = np.argsort(gid, kind="stable")
    gid_s = gid[order]
    counts = np.bincount(gid_s, minlength=NCORES * W * 2).reshape(NCORES, W, 2)

    # SPMD-uniform subtile counts per (window, half)
    ntiles = (counts.max(axis=0) + P - 1) // P  # [W, 2]
    empty = ntiles.sum(axis=1) == 0
    ntiles[empty, 0] = 1
    TL = int(ntiles[:, 0].sum())
    TH = int(ntiles[:, 1].sum())
    startA = np.concatenate([[0], np.cumsum(ntiles[:, 0])[:-1]]).astype(np.int64)
    startB = np.concatenate([[0], np.cumsum(ntiles[:, 1])[:-1]]).astype(np.int64)

    # per-core stream arrays
    idxA = np.zeros((NCORES, P, TL), np.int16)
    idxB = np.zeros((NCORES, P, TH), np.int16)
    relA = np.full((NCORES, P, TL), -1.0, np.float32)
    relB = np.full((NCORES, P, TH), -1.0, np.float32)
    invA = np.zeros((NCORES, P, TL), np.float32)
    invB = np.zeros((NCORES, P, TH), np.float32)

    # vectorized placement: rank of each edge within its (core, win, half) group
    grp_start_per_edge = np.concatenate([[0], np.cumsum(np.bincount(
        gid_s, minlength=NCORES * W * 2))])[gid_s]
    rank = np.arange(len(gid_s)) - grp_start_per_edge
    e_core = core[order]
    e_win = win[order]
    e_half = half[order]
    e_src = src[order]
    e_rel = rel[order]
    e_inv = invdeg_all[dst[order]]
    pos = np.where(e_half == 0, startA[e_win], startB[e_win]) + (rank >> 7)
    prt = rank & 127

    mA = e_half == 0
    idxA[e_core[mA], prt[mA], pos[mA]] = e_src[mA].astype(np.int16)
    relA[e_core[mA], prt[mA], pos[mA]] = e_rel[mA]
    invA[e_core[mA], prt[mA], pos[mA]] = e_inv[mA]
    mB = ~mA
    idxB[e_core[mB], prt[mB], pos[mB]] = (e_src[mB] - HALF).astype(np.int16)
    relB[e_core[mB], prt[mB], pos[mB]] = e_rel[mB]
    invB[e_core[mB], prt[mB], pos[mB]] = e_inv[mB]

    def wrap(idx_pt):  # [P, T] slot-major -> wrapped [128, T*8] per chunk
        Tn = idx_pt.shape[1]
        out = np.zeros((128, Tn * 8), np.int16)
        for c0 in range(0, Tn, G):
            c1 = min(c0 + G, Tn)
            flat = idx_pt[:, c0:c1].T.reshape(-1)  # i = t_local*128 + p
            w16 = flat.reshape(-1, 16).T  # [16, n/16]: i -> [i%16, i//16]
            out[:, c0 * 8 : c0 * 8 + w16.shape[1]] = np.tile(w16, (8, 1))
        return out

    idxAw = np.stack([wrap(idxA[c]) for c in range(NCORES)])
    idxBw = np.stack([wrap(idxB[c]) for c in range(NCORES)])

    def onehot(rel, inv):
        # [NCORES, P, T] -> [NCORES, P, T*P] fp16: oh[c,p,t*P+j] =
        # inv[c,p,t] * (j == rel[c,p,t]); rel<0 slots stay zero.
        nc_, pp, tt = rel.shape
        oh = np.zeros((nc_, pp, tt, P), np.float16)
        ci, pi, ti = np.nonzero(rel >= 0)
        oh[ci, pi, ti, rel[ci, pi, ti].astype(np.int64)] = inv[ci, pi, ti]
        return oh.reshape(nc_, pp, tt * P)

    return dict(
        ntiles=ntiles, TL=TL, TH=TH, startA=startA, startB=startB,
        idxA=idxAw, idxB=idxBw,
        ohA=onehot(relA, invA), ohB=onehot(relB, invB),
    )


def _build(sched):
    """Build and compile the SPMD Bass program."""
    ntiles, TL, TH = sched["ntiles"], sched["TL"], sched["TH"]
    startA, startB = sched["startA"], sched["startB"]

    nc = bacc.Bacc("TRN2", target_bir_lowering=False, debug=False,
                   num_devices=NCORES, num_swdge_queues=4,
                   dynamic_dma_scratch_size=65536)

    def din(name, shape, dt):
        return nc.dram_tensor(name, shape, dt, kind="ExternalInput")

    xf16_d = din("xf16", [N, D], f16)
    xfm16_d = din("xfm16", [P, NPAD], f16)
    idxA_d = din("idxA", [P, TL * 8], i16)
    idxB_d = din("idxB", [P, TH * 8], i16)
    ohA_d = din("ohA", [P, TL * P], f16)
    ohB_d = din("ohB", [P, TH * P], f16)
    ident_d = din("ident", [P, P], f16)
    statsel_d = din("statsel", [P, 64], f16)   # stats row-routing lhsT
    selbank_d = din("selbank", [P, 8 * P], f16)  # broadcast row-select lhsT
    magic_d = din("magic", [P, P], i32)     # rsqrt bit-hack constant
    c15_d = din("c15", [P, P], f32)         # Newton constant 1.5
    wl_d = [din(f"wl{l}", [D, D if l < 2 else DOUT], f16) for l in range(3)]
    wr_d = [din(f"wr{l}", [D, D if l < 2 else DOUT], f16) for l in range(3)]
    bl_d = [din(f"bl{l}", [D if l < 2 else DOUT, 1], f32) for l in range(3)]
    g_d = [din(f"g{l}", [D, 1], f32) for l in range(2)]
    b_d = [din(f"b{l}", [D, 1], f32) for l in range(2)]
    wc_d = din("wc", [DOUT, NCLS], f16)
    bc_d = din("bc", [NCLS, 1], f32)
    out_d = nc.dram_tensor("out", [NCLS, NPAD], f32, kind="ExternalOutput")

    xg_own = [nc.dram_tensor(f"xg{l}_own", [NPC, D], f16) for l in range(2)]
    xg_full = [
        nc.dram_tensor(f"xg{l}_full", [N, D], f16, addr_space="Shared")
        for l in range(2)
    ]

    with tile.TileContext(nc) as tc, ExitStack() as ctx:
        cpool = ctx.enter_context(tc.tile_pool(name="const", bufs=1))
        xpool = ctx.enter_context(tc.tile_pool(name="x", bufs=1))
        msgApool = ctx.enter_context(tc.tile_pool(name="msgA", bufs=4))
        msgBpool = ctx.enter_context(tc.tile_pool(name="msgB", bufs=4))
        ohApool = ctx.enter_context(tc.tile_pool(name="ohA", bufs=4))
        ohBpool = ctx.enter_context(tc.tile_pool(name="ohB", bufs=4))
        wkpool = ctx.enter_context(tc.tile_pool(name="wk", bufs=4))
        hsqpool = ctx.enter_context(tc.tile_pool(name="hsq", bufs=17))
        stpool = ctx.enter_context(tc.tile_pool(name="st", bufs=4))
        psA = ctx.enter_context(tc.tile_pool(name="psA", bufs=2, space="PSUM"))
        psH = ctx.enter_context(tc.tile_pool(name="psH", bufs=2, space="PSUM"))
        psS = ctx.enter_context(tc.tile_pool(name="psS", bufs=1, space="PSUM"))
        psB = ctx.enter_context(tc.tile_pool(name="psB", bufs=1, space="PSUM"))
        psT = ctx.enter_context(tc.tile_pool(name="psT", bufs=1, space="PSUM"))

        def load(dram, shape, dt):
            t = cpool.tile(shape, dt, name=f"c_{dram.name}")
            nc.sync.dma_start(out=t[:], in_=dram.ap())
            return t

        ident_t = load(ident_d, [P, P], f16)
        statsel_t = load(statsel_d, [P, 64], f16)
        selbank_t = load(selbank_d, [P, 8 * P], f16)
        magic_t = load(magic_d, [P, P], i32)
        c15_t = load(c15_d, [P, P], f32)
        wl_t = [load(wl_d[l], [D, D if l < 2 else DOUT], f16) for l in range(3)]
        wr_t = [load(wr_d[l], [D, D if l < 2 else DOUT], f16) for l in range(3)]
        bl_t = [load(bl_d[l], [D if l < 2 else DOUT, 1], f32) for l in range(3)]
        g_t = [load(g_d[l], [D, 1], f32) for l in range(2)]
        b_t = [load(b_d[l], [D, 1], f32) for l in range(2)]
        wc_t = load(wc_d, [DOUT, NCLS], f16)
        bc_t = load(bc_d, [NCLS, 1], f32)
        # layer-invariant gather indices, preloaded whole
        idxA_t = load(idxA_d, [P, TL * 8], i16)
        idxB_t = load(idxB_d, [P, TH * 8], i16)

        xfm16 = [xpool.tile([P, NPAD], f16, tag=f"xfm16_{i}", name=f"xfm16_{i}")
                 for i in range(2)]
        nc.sync.dma_start(out=xfm16[0][:], in_=xfm16_d.ap())

        qctr = [0]
        dma_sems = [nc.alloc_semaphore(f"swdge_dma{q}") for q in range(4)]

        for l in range(3):
            dout = D if l < 2 else DOUT
            cur16 = xfm16[l % 2]
            nxt16 = xfm16[(l + 1) % 2]
            if l == 0:
                src_lo = xf16_d.ap()[:HALF, :]
                src_hi = xf16_d.ap()[HALF:, :]
            else:
                src_lo = xg_full[l - 1].ap()[:HALF, :]
                src_hi = xg_full[l - 1].ap()[HALF:, :]

            # emit gather chunks lazily; Tile pool backpressure pipelines them
            msgs = {"A": {}, "B": {}}
            ohs = {"A": {}, "B": {}}
            issued = {"A": -1, "B": -1}

            def emit_chunk(stream, ci, src_lo=src_lo, src_hi=src_hi,
                           msgs=msgs, ohs=ohs):
                Tn = TL if stream == "A" else TH
                idxt = idxA_t if stream == "A" else idxB_t
                ohd = ohA_d if stream == "A" else ohB_d
                mpool = msgApool if stream == "A" else msgBpool
                opool = ohApool if stream == "A" else ohBpool
                src = src_lo if stream == "A" else src_hi
                c0 = ci * G
                cn = min(G, Tn - c0)
                nidx = cn * P
                q = qctr[0] % 4
                mt = mpool.tile([P, G * P], f16, tag=f"msg{stream}")
                nc.gpsimd.dma_gather(
                    mt[:, : cn * P].rearrange("p (t d) -> p t d", d=P),
                    src,
                    idxt[:, c0 * 8 : c0 * 8 + cn * 8],
                    nidx,
                    nidx,
                    P,
                    single_packet=False,
                    queue_num=q,
                )
                qctr[0] += 1
                ot = opool.tile([P, G * P], f16, tag=f"oh{stream}")
                nc.sync.dma_start(
                    out=ot[:, : cn * P],
                    in_=ohd.ap()[:, c0 * P : (c0 + cn) * P],
                )
                msgs[stream][ci] = mt
                ohs[stream][ci] = ot

            def phase1(w, blk_i, kb, sps_blk, hsqs):
                """agg + SAGE linear + [h | h^2] + stats matmul for window w."""
                nA, nB = int(ntiles[w, 0]), int(ntiles[w, 1])
                subs = [("A", int(startA[w]) + i) for i in range(nA)] + [
                    ("B", int(startB[w]) + i) for i in range(nB)
                ]
                for stream, pos in subs:
                    while issued[stream] < pos // G:
                        issued[stream] += 1
                        emit_chunk(stream, issued[stream])

                cols = slice(w * P, (w + 1) * P)
                ps = psA.tile([P, P], f32, space="PSUM", tag="agg")
                for si, (stream, pos) in enumerate(subs):
                    mt = msgs[stream][pos // G]
                    ot = ohs[stream][pos // G]
                    t = pos % G
                    nc.tensor.matmul(
                        out=ps[:],
                        lhsT=mt[:, t * P : (t + 1) * P],
                        rhs=ot[:, t * P : (t + 1) * P],
                        start=(si == 0),
                        stop=(si == len(subs) - 1),
                    )
                agg16 = wkpool.tile([P, P], f16, tag="agg16")
                nc.any.tensor_copy(agg16[:], ps[:])

                hps = psH.tile([dout, P], f32, space="PSUM", tag="h")
                nc.tensor.matmul(out=hps[:], lhsT=wl_t[l][:], rhs=agg16[:],
                                 start=True, stop=False)
                nc.tensor.matmul(out=hps[:], lhsT=wr_t[l][:],
                                 rhs=cur16[:, cols],
                                 start=False, stop=True)

                # hsq = [h | h^2] fp16; ACT stays in the gelu_and_others
                # table set (Identity/Square/Gelu) -- no table reloads.
                hsq = hsqpool.tile([dout, 2 * P], f16, tag="hsq")
                fn1 = (mybir.ActivationFunctionType.Identity if l < 2
                       else mybir.ActivationFunctionType.Gelu)
                nc.scalar.activation(hsq[:, :P], hps[:], fn1,
                                     bias=bl_t[l][:, :1])
                nc.scalar.activation(hsq[:, P:], hsq[:, :P],
                                     mybir.ActivationFunctionType.Square)
                # route this window's [sum | sumsq] into block row blk_i:
                # lhsT column j is all-ones iff j == blk_i
                nc.tensor.matmul(out=sps_blk[:],
                                 lhsT=statsel_t[:dout, blk_i * 8 : blk_i * 8 + 8],
                                 rhs=hsq[:], start=(blk_i == 0),
                                 stop=(blk_i == kb - 1))
                hsqs[w] = hsq

            def strip_math(kb, sps_blk):
                """per-node LN stats -> [a | b2] for a block of kb windows.
                All fast-path DVE ops (single float scalars / tensor-tensor);
                rstd via bit-hack seed + 2 Newton iterations."""
                invd = 1.0 / dout
                sl = slice(0, kb)
                ex2 = stpool.tile([kb, P], f32, tag="ex2")
                nc.vector.tensor_scalar_mul(ex2[:], sps_blk[sl, P:], invd)
                negmu = stpool.tile([kb, P], f32, tag="negmu")
                nc.vector.tensor_scalar_mul(negmu[:], sps_blk[sl, :P], -invd)
                musq = stpool.tile([kb, P], f32, tag="musq")
                nc.vector.tensor_mul(musq[:], negmu[:], negmu[:])
                musqe = stpool.tile([kb, P], f32, tag="musqe")
                nc.vector.tensor_scalar_sub(musqe[:], musq[:], LN_EPS)
                vare = stpool.tile([kb, P], f32, tag="vare")
                nc.vector.tensor_sub(vare[:], ex2[:], musqe[:])
                ishr = stpool.tile([kb, P], i32, tag="ishr")
                nc.vector.tensor_scalar(
                    ishr[:], vare[:].bitcast(i32), 1, None,
                    op0=mybir.AluOpType.logical_shift_right)
                y0 = stpool.tile([kb, P], i32, tag="y0")
                nc.vector.tensor_sub(y0[:], magic_t[:kb, :P], ishr[:])
                halfv = stpool.tile([kb, P], f32, tag="halfv")
                nc.vector.tensor_scalar_mul(halfv[:], vare[:], 0.5)
                ab = stpool.tile([kb, 2 * P], f16, tag="ab")
                ycur = y0[:].bitcast(f32)
                for it in range(2):
                    yy = stpool.tile([kb, P], f32, tag=f"yy{it}")
                    nc.vector.tensor_mul(yy[:], ycur, ycur)
                    t_ = stpool.tile([kb, P], f32, tag=f"t{it}")
                    nc.vector.tensor_mul(t_[:], halfv[:], yy[:])
                    s_ = stpool.tile([kb, P], f32, tag=f"s{it}")
                    nc.vector.tensor_sub(s_[:], c15_t[:kb, :P], t_[:])
                    if it == 0:
                        yn = stpool.tile([kb, P], f32, tag="y1")
                        nc.vector.tensor_mul(yn[:], ycur, s_[:])
                        ycur = yn[:]
                    else:
                        nc.vector.tensor_mul(ab[:, :P], ycur, s_[:])
                nc.vector.tensor_mul(ab[:, P:], negmu[:], ab[:, :P])
                return ab

            def phase3(w, blk_i, kb, ab, hsqs):
                """broadcast + normalize + gelu (+ residual / classifier)."""
                cols = slice(w * P, (w + 1) * P)
                hsq = hsqs.pop(w)
                bps = psB.tile([dout, 2 * P], f32, space="PSUM", tag="bc")
                nc.tensor.matmul(out=bps[:],
                                 lhsT=selbank_t[:kb, blk_i * P : blk_i * P + dout],
                                 rhs=ab[:kb, :],
                                 start=True, stop=True)
                t1 = wkpool.tile([dout, P], f32, tag="t1")
                nc.vector.tensor_mul(t1[:], hsq[:, :P], bps[:, :P])
                if l < 2:
                    t2 = wkpool.tile([dout, P], f32, tag="t2")
                    nc.vector.tensor_add(t2[:], t1[:], bps[:, P:])
                    gel16 = wkpool.tile([dout, P], f16, tag="gel16")
                    nc.scalar.activation(gel16[:], t2[:],
                                         mybir.ActivationFunctionType.Gelu,
                                         bias=b_t[l][:, :1],
                                         scale=g_t[l][:, :1])
                    nc.vector.tensor_add(nxt16[:, cols], gel16[:],
                                         cur16[:, cols])
                    tp = psT.tile([P, P], f16, space="PSUM", tag="tp")
                    nc.tensor.transpose(tp[:], nxt16[:, cols], ident_t[:])
                    xnm = wkpool.tile([P, P], f16, tag="xnm")
                    nc.any.tensor_copy(xnm[:], tp[:])
                    rows = min(P, NPC - w * P)
                    nc.sync.dma_start(
                        out=xg_own[l].ap()[w * P : w * P + rows, :],
                        in_=xnm[:rows, :],
                    )
                else:
                    norm16 = wkpool.tile([dout, P], f16, tag="norm16")
                    nc.vector.tensor_add(norm16[:], t1[:], bps[:, P:])
                    ops_ = psB.tile([NCLS, P], f32, space="PSUM", tag="bc")
                    nc.tensor.matmul(out=ops_[:], lhsT=wc_t[:],
                                     rhs=norm16[:], start=True, stop=True)
                    osb = wkpool.tile([NCLS, P], f32, tag="osb")
                    nc.scalar.activation(osb[:], ops_[:],
                                         mybir.ActivationFunctionType.Identity,
                                         bias=bc_t[:, :1])
                    nc.sync.dma_start(out=out_d.ap()[:, cols], in_=osb[:])

            # software-pipelined blocks: phase1(b), phase3(b-1), strip(b)
            BK = 8
            blocks = [list(range(b, min(b + BK, W))) for b in range(0, W, BK)]
            hsqs = {}
            prev = None  # (wins, ab)
            for wins in blocks:
                kb = len(wins)
                sps_blk = psS.tile([BK, 2 * P], f32, space="PSUM", tag="st")
                for i, w in enumerate(wins):
                    phase1(w, i, kb, sps_blk, hsqs)
                if prev is not None:
                    pwins, pab = prev
                    for i, w in enumerate(pwins):
                        phase3(w, i, len(pwins), pab, hsqs)
                prev = (wins, strip_math(kb, sps_blk))
            pwins, pab = prev
            for i, w in enumerate(pwins):
                phase3(w, i, len(pwins), pab, hsqs)

            if l < 2:
                nc.gpsimd.collective_compute(
                    "AllGather",
                    mybir.AluOpType.bypass,
                    replica_groups=[list(range(NCORES))],
                    ins=[xg_own[l].ap()],
                    outs=[xg_full[l].ap()],
                )

    nc.compile()
    return nc


def _statsel():
    # statsel[p, i*8 + j] = 1 iff j == i: lhsT slice [dout, 8] for window i
    # has column i all-ones -> stats land in block row i.
    s = np.zeros((P, 64), np.float16)
    for i in range(8):
        s[:, i * 8 + i] = 1.0
    return s


def _selbank():
    # selbank[j, i*P + f] = 1 iff j == i: lhsT slice [kb, dout] for window i
    # selects block-strip row i and broadcasts it across all dout partitions.
    s = np.zeros((P, 8 * P), np.float16)
    for i in range(8):
        s[i, i * P : (i + 1) * P] = 1.0
    return s


def _prep_inputs(x, sched, weights):
    """Build per-core input maps."""
    xf16 = x.astype(np.float16)
    (Wl1, bl1, Wr1, g1, b1, Wl2, bl2, Wr2, g2, b2,
     Wl3, bl3, Wr3, gc, bc, Wc, bcls) = weights
    wcp = (gc[:, None].astype(np.float32) * Wc.astype(np.float32))
    bcp = bc.astype(np.float32) @ Wc.astype(np.float32) + bcls.astype(np.float32)
    common = {
        "xf16": xf16,
        "ident": np.eye(P, dtype=np.float16),
        "statsel": _statsel(),
        "selbank": _selbank(),
        "magic": np.full((P, P), RSQRT_MAGIC, np.int32),
        "c15": np.full((P, P), 1.5, np.float32),
        "wl0": Wl1.astype(np.float16), "wr0": Wr1.astype(np.float16),
        "wl1": Wl2.astype(np.float16), "wr1": Wr2.astype(np.float16),
        "wl2": Wl3.astype(np.float16), "wr2": Wr3.astype(np.float16),
        "bl0": bl1.reshape(-1, 1).astype(np.float32),
        "bl1": bl2.reshape(-1, 1).astype(np.float32),
        "bl2": bl3.reshape(-1, 1).astype(np.float32),
        "g0": g1.reshape(-1, 1).astype(np.float32),
        "b0": b1.reshape(-1, 1).astype(np.float32),
        "g1": g2.reshape(-1, 1).astype(np.float32),
        "b1": b2.reshape(-1, 1).astype(np.float32),
        "wc": wcp.astype(np.float16),
        "bc": bcp.reshape(-1, 1).astype(np.float32),
    }
    in_maps = []
    for c in range(NCORES):
        xc_ = x[c * NPC : (c + 1) * NPC].astype(np.float16)
        xfm = np.zeros((P, NPAD), np.float16)
        xfm[:, :NPC] = xc_.T
        m = dict(common)
        m.update(
            xfm16=xfm,
            idxA=sched["idxA"][c],
            idxB=sched["idxB"][c],
            ohA=sched["ohA"][c],
            ohB=sched["ohB"][c],
        )
        in_maps.append(m)
    return in_maps




class _Runner:
    """Persistent PJRT runner: traces/compiles once, keeps inputs on device,
    supports steady-state timing of repeated executions."""

    def __init__(self, nc, in_maps):
        import jax
        from jax.sharding import Mesh, PartitionSpec
        try:
            from jax.experimental.shard_map import shard_map
        except ImportError:
            from jax.shard_map import shard_map
        from concourse import bass2jax, mybir as mb

        bass2jax.install_neuronx_cc_hook()
        self.jax = jax
        partition_name = (
            nc.partition_id_tensor.name if nc.partition_id_tensor else None
        )
        in_names, out_names, out_avals, zero_outs = [], [], [], []
        for alloc in nc.m.functions[0].allocations:
            if not isinstance(alloc, mb.MemoryLocationSet):
                continue
            name = alloc.memorylocations[0].name
            if alloc.kind == "ExternalInput":
                if name != partition_name:
                    in_names.append(name)
            elif alloc.kind == "ExternalOutput":
                out_names.append(name)
                shape = tuple(alloc.tensor_shape)
                dtype = mb.dt.np(alloc.dtype)
                out_avals.append(jax.core.ShapedArray(shape, dtype))
                zero_outs.append(np.zeros(shape, dtype))
        n_params = len(in_names)
        all_names = in_names + out_names
        if partition_name is not None:
            all_names.append(partition_name)

        def _body(*args):
            operands = list(args)
            if partition_name is not None:
                operands.append(bass2jax.partition_id_tensor())
            outs = bass2jax._bass_exec_p.bind(
                *operands,
                out_avals=tuple(out_avals),
                in_names=tuple(all_names),
                out_names=tuple(out_names),
                lowering_input_output_aliases=(),
                sim_require_finite=True,
                sim_require_nnan=True,
                nc=nc,
            )
            return tuple(outs)

        devices = jax.devices()[:NCORES]
        mesh = Mesh(np.asarray(devices), ("core",))
        n_outs = len(out_avals)
        self.fn = jax.jit(
            shard_map(
                _body,
                mesh=mesh,
                in_specs=(PartitionSpec("core"),) * (n_params + n_outs),
                out_specs=(PartitionSpec("core"),) * n_outs,
                check_rep=False,
            ),
            keep_unused=True,
        )
        self.out_names = out_names
        self.out_avals = out_avals
        concat_in = [
            np.concatenate([np.asarray(in_maps[c][nm]) for c in range(NCORES)])
            for nm in in_names
        ]
        concat_zeros = [
            np.concatenate([z] * NCORES, axis=0) for z in zero_outs
        ]
        self.dev_args = [jax.device_put(a) for a in concat_in + concat_zeros]
        self.update_idx = {nm: i for i, nm in enumerate(in_names)}
        self.in_names = in_names

    def refresh(self, in_maps):
        for nm in self.in_names:
            arr = np.concatenate(
                [np.asarray(in_maps[c][nm]) for c in range(NCORES)]
            )
            self.dev_args[self.update_idx[nm]] = self.jax.device_put(arr)

    def update_input(self, name, per_core_arrays):
        arr = np.concatenate([np.asarray(a) for a in per_core_arrays])
        self.dev_args[self.update_idx[name]] = self.jax.device_put(arr)

    def run(self):
        outs = self.fn(*self.dev_args)
        self.jax.block_until_ready(outs)
        return [
            {
                nm: np.asarray(outs[i]).reshape(NCORES, *self.out_avals[i].shape)[c]
                for i, nm in enumerate(self.out_names)
            }
            for c in range(NCORES)
        ]

    def time(self, reps=20, warmup=2):
        import time as _time
        for _ in range(warmup):
            self.jax.block_until_ready(self.fn(*self.dev_args))
        t0 = _time.time()
        outs = None
        for _ in range(reps):
            outs = self.fn(*self.dev_args)
        self.jax.block_until_ready(outs)
        return (_time.time() - t0) / reps


def kernel(x, edge_index, Wl1, bl1, Wr1, g1, b1, Wl2, bl2, Wr2, g2, b2,
           Wl3, bl3, Wr3, gc, bc, Wc, bcls):
    x = np.asarray(x)
    edge_index = np.asarray(edge_index)
    runner = get_runner(x, edge_index, Wl1, bl1, Wr1, g1, b1, Wl2, bl2, Wr2,
                        g2, b2, Wl3, bl3, Wr3, gc, bc, Wc, bcls)
    results = runner.run()
    out = np.empty((N, NCLS), np.float32)
    for c in range(NCORES):
        out[c * NPC : (c + 1) * NPC] = results[c]["out"][:, :NPC].T
    return out


def get_runner(x, edge_index, Wl1, bl1, Wr1, g1, b1, Wl2, bl2, Wr2, g2, b2,
               Wl3, bl3, Wr3, gc, bc, Wc, bcls):
    x = np.asarray(x)
    edge_index = np.asarray(edge_index)
    sched = _schedule(edge_index)
    key = (sched["TL"], sched["TH"], tuple(sched["ntiles"].ravel().tolist()))
    if key not in _cache:
        _cache[key] = _build(sched)
    nc = _cache[key]
    weights = (Wl1, bl1, Wr1, g1, b1, Wl2, bl2, Wr2, g2, b2,
               Wl3, bl3, Wr3, gc, bc, Wc, bcls)
    in_maps = _prep_inputs(x, sched, [np.asarray(w) for w in weights])
    rkey = ("runner", key)
    if rkey not in _cache:
        _cache[rkey] = _Runner(nc, in_maps)
    else:
        _cache[rkey].refresh(in_maps)
    return _cache[rkey]
